# revision 20
# baseline (speedup 1.0000x reference)
"""Causal self-attention (B=4, N=2048, D=1024, single head) on 8 TRN2 NeuronCores.

Sharding: core c handles batch b = c//2, query shard h = c%2 with the
stride-2 interleave q_global = 2*j + h  (j = 0..1023).  The interleave makes
the causal-mask *tile structure* identical on every core (SPMD-uniform), so
fully-masked score tiles are skipped structurally while the residual
diagonal masking is data-driven (query-position tensor per core).

Because the attention is single-head (D_head == D_model), the four weight
matrices fold into two host-side products, removing the K and V projections
entirely:
  scores ~ Xq @ (Wq^T Wk) @ X^T + (Wk^T bq).X^T   (+ per-query terms that
                                                   softmax ignores)
  out    = [P @ X] @ (Wo Wv)^T / rowsum + (bo + Wo bv)

Per-core pipeline (f32 PSUM accumulation everywhere):
  GT[e,q]  = W_qk^T @ Xq + bgt   ct-outer accumulation, one pass per query
                                 half, so matmuls start as soon as the first
                                 (wqk, xtq) DMA tiles land; gt tiles are
                                 split per query-half so ST jc0 does not
                                 depend on the second pass's evictions
  ST[k,j]  = X^T-pairs @ GT      (fp8 DoubleRow; half-width for the score
                                 tiles whose lower query half is fully
                                 masked -- their E halves are pre-zeroed)
  E        = exp(ST/sqrt(D)); boundary masks applied in-place on DVE
  rowsum[j]= ones.T @ E          (PE reduction over k partitions)
  Z[e,j]   = X^T @ E             (jc0 split into per-half column groups to
                                 skip the masked-out region; evict * 1/rowsum)
  OT[e,j]  = W_vo^T @ Z          per 512-chunk; Identity+bias eviction on the
                                 Act engine, fp16 output DMA on the Sync queue
                                 starting ~60% into the kernel

Scheduling rules learned from traces: DMA-issue instructions block the
issuing engine queue until the ring accepts them, so bulk loads only go on
queues ahead of that engine's compute needs (Act gets only what it needs
before its first exp); dependencies are tile-granular, so anything consumed
at finer granularity gets its own tile; dummy warm-up matmuls ramp the PE
DVFS clock during the initial DMA wait.  1/rowsum via reciprocal_approx_fast.
No collectives: each core receives exactly the host-side shard it needs.
"""

import os
import numpy as np
import ml_dtypes

BF16 = ml_dtypes.bfloat16
FP8 = ml_dtypes.float8_e4m3
F16 = np.float16

N_CORES = 8
B, N, D = 4, 2048, 1024
NQ = 1024           # queries per core
P = 128             # partitions
ET = D // P         # 8  e-tiles
CT_ = D // P        # 8  contraction tiles of D
KT_ALL = N // P     # 16 key tiles
JCW = 512           # query chunk
NJC = NQ // JCW     # 2
QCW = 256           # query half-chunk (causal granularity)

_cache = {}


def _build():
    from concourse import bacc, tile, mybir
    import concourse.bass as bass

    f32 = mybir.dt.float32
    bf16 = mybir.dt.bfloat16
    fp8 = mybir.dt.float8e4
    fp16 = mybir.dt.float16
    DR = mybir.MatmulPerfMode.DoubleRow
    Exp = mybir.ActivationFunctionType.Exp
    Ident = mybir.ActivationFunctionType.Identity
    is_ge = mybir.AluOpType.is_ge
    mult = mybir.AluOpType.mult
    PSUM = bass.MemorySpace.PSUM

    SCL = float(1.0 / np.sqrt(np.float32(D)))
    nc = bacc.Bacc("TRN2", target_bir_lowering=False, debug=False,
                   num_devices=N_CORES)

    xtp_d = nc.declare_dram_parameter("xtp", [ET // 2, P, 2, N], fp8,
                                      isOutput=False)
    xtq_d = nc.declare_dram_parameter("xtq", [D, NQ], bf16, isOutput=False)
    wqk_d = nc.declare_dram_parameter("wqk", [D, D], bf16, isOutput=False)
    wvot_d = nc.declare_dram_parameter("wvot", [D, D], bf16, isOutput=False)
    xtok_d = nc.declare_dram_parameter("xtok", [N, D], bf16, isOutput=False)
    bgt_d = nc.declare_dram_parameter("bgt", [P, ET], f32, isOutput=False)
    bot_d = nc.declare_dram_parameter("bot", [P, ET], f32, isOutput=False)
    bqp_d = nc.declare_dram_parameter("bqpos", [P, NQ], f32, isOutput=False)
    kpt_d = nc.declare_dram_parameter("kpost", [P, KT_ALL], f32, isOutput=False)
    out_d = nc.declare_dram_parameter("out", [D, NQ], fp16, isOutput=True)

    # Causal structure with the stride-2 query interleave: keys of tile kt
    # are visible only to query quarter-chunks qc >= kt//4.  Within a
    # 512-wide chunk jc, tiles whose lower quarter is fully masked are
    # computed half-width; the dead quarter of E is pre-zeroed instead.
    def dead_left(jc, kt):
        return kt >= 4 * (2 * jc + 1)

    with tile.TileContext(nc) as tc:
        with (
            tc.tile_pool(name="consts", bufs=1) as p_c,
            tc.tile_pool(name="wq", bufs=CT_) as p_wq,
            tc.tile_pool(name="wo", bufs=CT_) as p_wo,
            tc.tile_pool(name="xtq", bufs=CT_) as p_xtq,
            tc.tile_pool(name="qt", bufs=ET) as p_qt,
            tc.tile_pool(name="kt", bufs=ET // 2) as p_kt,
            tc.tile_pool(name="v", bufs=KT_ALL) as p_v,
            tc.tile_pool(name="exp", bufs=NJC * KT_ALL - ET) as p_exp,
            tc.tile_pool(name="z", bufs=NJC * CT_) as p_z,
            tc.tile_pool(name="recip", bufs=4) as p_recip,
            tc.tile_pool(name="brec", bufs=2) as p_brec,
            tc.tile_pool(name="of", bufs=6) as p_of,
        ):
            # gt tiles split per (d-pair, query-half) for fine-grained deps
            gtq = {(i, qh): p_qt.tile([P, 2, JCW], fp8, tag="qt", name="qt")
                   for i in range(ET // 2) for qh in range(NJC)}
            xtp_tiles = [p_kt.tile([P, 2, N], fp8, tag="kt", name="kt")
                         for _ in range(ET // 2)]
            xtok_tiles = [p_v.tile([P, D], bf16, tag="v", name="v")
                          for _ in range(KT_ALL)]
            exps = {(jc, kt): p_exp.tile([P, JCW], bf16, tag="exp",
                                         name="exp")
                    for jc in range(NJC) for kt in range(KT_ALL)
                    if kt < (8 if jc == 0 else 16)}

            # t=0 work on DVE: constants + zero the dead E halves so the
            # full-width rowsum matmuls read zeros there
            ones_col = p_c.tile([P, 1], bf16, tag="ones_col")
            nc.vector.memset(ones_col[:], 1.0)
            ones_row = p_c.tile([1, P], bf16, tag="ones_row")
            nc.vector.memset(ones_row[:], 1.0)
            dummy_b = p_c.tile([P, JCW], bf16, tag="dummy")
            nc.vector.memset(dummy_b[:], 1.0)
            for jc in range(NJC):
                for kt in range(KT_ALL):
                    if (jc, kt) in exps and dead_left(jc, kt):
                        nc.vector.memset(exps[(jc, kt)][:, :QCW], 0.0)

            # ---- DMA issue plan.  Act (scalar) only gets what must land
            # before its first compute; Sync carries the early weights and
            # half the late bulk + outputs; gpsimd (software DGE) the rest.
            wq = [p_wq.tile([P, D], bf16, tag="wq", name="wq")
                  for _ in range(CT_)]
            xtq_tiles = [p_xtq.tile([P, NQ], bf16, tag="xtq", name="xtq")
                         for _ in range(CT_)]
            wo = [p_wo.tile([P, D], bf16, tag="wo", name="wo")
                  for _ in range(CT_)]

            def load(eng, t, dram, r0):
                eng.dma_start(t[:], dram[r0 * P:(r0 + 1) * P, :])

            bgt_t = p_c.tile([P, ET], f32, tag="bgt")
            nc.sync.dma_start(bgt_t[:], bgt_d[:, :])
            pair_q = {0: nc.sync, 1: nc.scalar, 2: nc.sync, 3: nc.scalar,
                      4: nc.sync, 5: nc.gpsimd, 6: nc.sync, 7: nc.gpsimd}
            for ct in range(CT_):
                load(pair_q[ct], wq[ct], wqk_d, ct)
                load(pair_q[ct], xtq_tiles[ct], xtq_d, ct)
            for i in range(ET // 2):
                eng = nc.scalar if i < 2 else nc.gpsimd
                eng.dma_start(xtp_tiles[i][:], xtp_d[i])
            bqpos_t = p_c.tile([P, NQ], f32, tag="bqpos")
            nc.gpsimd.dma_start(bqpos_t[:], bqp_d[:, :])
            kpost_t = p_c.tile([P, KT_ALL], f32, tag="kpost")
            nc.gpsimd.dma_start(kpost_t[:], kpt_d[:, :])
            for kt in range(KT_ALL):
                eng = nc.sync if kt % 2 == 0 else nc.gpsimd
                load(eng, xtok_tiles[kt], xtok_d, kt)
            for ct in range(CT_):
                eng = nc.sync if ct % 2 == 0 else nc.gpsimd
                load(eng, wo[ct], wvot_d, ct)
            bot_t = p_c.tile([P, ET], f32, tag="bot")
            nc.gpsimd.dma_start(bot_t[:], bot_d[:, :])

            # ---- GT = W_qk^T @ Xq + bgt : ct-outer accumulation in
            # PSUM so matmuls start as soon as (wq[ct], xtq[ct]) land.
            # One pass per query-half: the qh=0 evictions finish during
            # the qh=1 compute pass, so ST jc0 starts with zero stall ----
            with tc.tile_pool(name="gtps", bufs=8, space=PSUM) as p_gt:
                # dummy matmuls ramp the PE clock while the first DMAs land
                dps = p_gt.tile([P, JCW], f32, tag="gtps", name="gtps")
                for _ in range(12):
                    nc.tensor.matmul(dps[0:1, :], ones_col[:, :],
                                     dummy_b[:], start=True, stop=True)
                for qh in range(NJC):
                    gtps = [p_gt.tile([P, JCW], f32, tag="gtps",
                                      name="gtps") for _ in range(ET)]
                    for ct in range(CT_):
                        for et in range(ET):
                            nc.tensor.matmul(
                                gtps[et][:],
                                wq[ct][:, et * P:(et + 1) * P],
                                xtq_tiles[ct][:, qh * JCW:(qh + 1) * JCW],
                                start=(ct == 0), stop=(ct == CT_ - 1))
                    for et in range(ET):
                        dst = gtq[(et // 2, qh)][:, et % 2, :]
                        if et % 2 == 0:
                            nc.scalar.activation(dst, gtps[et][:], Ident,
                                                 bias=bgt_t[:, et:et + 1])
                        else:
                            nc.vector.tensor_scalar_add(
                                dst, gtps[et][:], bgt_t[:, et:et + 1])

            with (
                tc.tile_pool(name="ps", bufs=2, space=PSUM) as p_ps,
                tc.tile_pool(name="ot", bufs=4, space=PSUM) as p_ot,
                tc.tile_pool(name="rsps", bufs=2, space=PSUM) as p_rs,
            ):
                # jc=0 covers global queries [0,1024): keys < 1024 (kt 0..7).
                # jc=1 covers [1024,2048): all 16 kt.
                rs_ps = {}
                nkt_of = {0: ET, 1: KT_ALL}

                def st_block(jc):
                    """ST + exp + mask + rowsum for one 512-wide chunk."""
                    nkt = nkt_of[jc]
                    rs_ps[jc] = p_rs.tile([1, JCW], f32, tag="rsps",
                                          name="rsps")
                    pend = []

                    def issue_rowsum(kt):
                        nc.tensor.matmul(
                            rs_ps[jc][:], ones_col[:], exps[(jc, kt)][:],
                            start=(kt == 0), stop=(kt == nkt - 1))

                    for kt in range(nkt):
                        dead = dead_left(jc, kt)
                        csl = slice(QCW, JCW) if dead else slice(0, JCW)
                        sts = p_ps.tile([P, JCW], f32, tag="ps", name="ps")
                        for i in range(ET // 2):
                            nc.tensor.matmul(
                                sts[:, csl],
                                xtp_tiles[i][:, :, kt * P:(kt + 1) * P],
                                gtq[(i, jc)][:, :, csl],
                                start=(i == 0), stop=(i == ET // 2 - 1),
                                perf_mode=DR)
                        ex_t = exps[(jc, kt)]
                        nc.scalar.activation(ex_t[:, csl], sts[:, csl],
                                             Exp, scale=SCL)
                        # in-place causal mask on the boundary quarter
                        for qh in range(2):
                            qc = 2 * jc + qh
                            if not (4 * qc <= kt < 4 * (qc + 1)):
                                continue
                            esl = slice(qh * QCW, (qh + 1) * QCW)
                            jsl = slice(jc * JCW + qh * QCW,
                                        jc * JCW + (qh + 1) * QCW)
                            nc.vector.scalar_tensor_tensor(
                                ex_t[:, esl], bqpos_t[:, jsl],
                                kpost_t[:, kt:kt + 1], ex_t[:, esl],
                                is_ge, mult)
                        pend.append(kt)
                        if len(pend) > 2:
                            issue_rowsum(pend.pop(0))
                    for kt in pend:
                        issue_rowsum(kt)

                def recip_block(jc):
                    r32 = p_recip.tile([1, JCW], f32, tag="recip32",
                                       name="recip32")
                    nc.vector.reciprocal_approx_fast(r32[:], rs_ps[jc][:])
                    r16 = p_recip.tile([1, JCW], bf16, tag="recip16",
                                       name="recip16")
                    nc.vector.tensor_copy(r16[:], r32[:])
                    return r16

                brec = {}

                def brec_block(jc, r16):
                    # broadcast 1/rowsum across partitions via K=1 matmul,
                    # then park it in SBUF so the PSUM bank frees right away
                    bps = p_ps.tile([P, JCW], f32, tag="ps", name="ps")
                    nc.tensor.matmul(bps[:], ones_row[:], r16[:],
                                     start=True, stop=True)
                    bt = p_brec.tile([P, JCW], f32, tag="brec", name="brec")
                    nc.vector.tensor_copy(bt[:], bps[:])
                    brec[jc] = bt

                zs = {}

                def zot_block(jc):
                    """Z then OT for one 512-chunk, with OT's first et-half
                    interleaved ct-outer into the Z loop so the PE never
                    waits on Z evictions, and output DMA starts early."""
                    jsl = slice(jc * JCW, (jc + 1) * JCW)
                    nkt = nkt_of[jc]
                    ot_ps = {}

                    def ot_mms(ct, ets):
                        for et in ets:
                            nc.tensor.matmul(
                                ot_ps[et][:],
                                wo[ct][:, et * P:(et + 1) * P],
                                zs[(jc, ct)][:],
                                start=(ct == 0), stop=(ct == CT_ - 1))

                    def ot_evict(ets):
                        for et in ets:
                            of = p_of.tile([P, JCW], fp16, tag="of",
                                           name="of")
                            nc.scalar.activation(of[:], ot_ps[et][:], Ident,
                                                 bias=bot_t[:, et:et + 1])
                            eng = nc.sync if et % 2 == 0 else nc.gpsimd
                            eng.dma_start(
                                out_d[et * P:(et + 1) * P, jsl], of[:])

                    ets1, ets2 = (0, 1, 2, 3), (4, 5, 6, 7)
                    for et in ets1:
                        ot_ps[et] = p_ot.tile([P, JCW], f32, tag="ot",
                                              name="ot")
                    for ct in range(CT_):
                        zps = p_ps.tile([P, JCW], f32, tag="ps", name="ps")
                        for kt in range(nkt):
                            csl = (slice(QCW, JCW) if dead_left(jc, kt)
                                   else slice(0, JCW))
                            nc.tensor.matmul(
                                zps[:, csl],
                                xtok_tiles[kt][:, ct * P:(ct + 1) * P],
                                exps[(jc, kt)][:, csl],
                                start=(kt == 0), stop=(kt == nkt - 1),
                                skip_group_check=True)
                        z_t = p_z.tile([P, JCW], bf16, tag="z", name="z")
                        nc.vector.tensor_tensor(z_t[:], zps[:],
                                                brec[jc][:], mult)
                        zs[(jc, ct)] = z_t
                        if ct >= 1:
                            ot_mms(ct - 1, ets1)
                    ot_mms(CT_ - 1, ets1)
                    ot_evict(ets1)
                    for et in ets2:
                        ot_ps[et] = p_ot.tile([P, JCW], f32, tag="ot",
                                              name="ot")
                    for ct in range(CT_):
                        ot_mms(ct, ets2)
                    ot_evict(ets2)

                # ---- PE program order; recip chains overlap the next block
                st_block(0)
                recip0 = recip_block(0)
                st_block(1)
                brec_block(0, recip0)
                recip1 = recip_block(1)
                brec_block(1, recip1)
                zot_block(0)
                zot_block(1)

    nc.compile()
    return nc


def _prep_in_maps(X, Wq, bq, Wk, bk, Wv, bv, Wo, bo):
    wqk = np.ascontiguousarray(Wq.astype(np.float64).T
                               @ Wk.astype(np.float64)).astype(BF16)
    wvot = np.ascontiguousarray((Wo.astype(np.float64)
                                 @ Wv.astype(np.float64)).T).astype(BF16)
    bgt = np.ascontiguousarray(
        (Wk.astype(np.float64).T @ bq.astype(np.float64))
        .reshape(ET, P).T).astype(np.float32)
    bo_eff = (bo.astype(np.float64)
              + Wo.astype(np.float64) @ bv.astype(np.float64))
    bot = np.ascontiguousarray(
        bo_eff.reshape(ET, P).T).astype(np.float32)
    kpost = np.ascontiguousarray(
        np.arange(N, dtype=np.float32).reshape(KT_ALL, P).T)

    in_maps = []
    for c in range(N_CORES):
        b, h = c // 2, c % 2
        Xb = X[b]
        xtok = np.ascontiguousarray(Xb).astype(BF16)
        xtq = np.ascontiguousarray(Xb[h::2].T).astype(BF16)
        xtp = np.ascontiguousarray(
            Xb.T.reshape(ET // 2, 2, P, N).transpose(0, 2, 1, 3)
        ).astype(FP8)
        qpos = (2.0 * np.arange(NQ, dtype=np.float32) + h)
        bqpos = np.ascontiguousarray(
            np.broadcast_to(qpos[None, :], (P, NQ))).astype(np.float32)
        in_maps.append({
            "xtp": xtp, "xtq": xtq, "xtok": xtok,
            "wqk": wqk, "wvot": wvot,
            "bgt": bgt, "bot": bot,
            "bqpos": bqpos, "kpost": kpost,
        })
    return in_maps


last_exec_time_ns = None


def _ensure_ntff_hook():
    """Register the axon NTFF profile hook if the image's antenv lacks it."""
    try:
        from antenv.axon_hooks import get_axon_ntff_profile_hook  # noqa: F401
        return
    except ImportError:
        pass
    import sys
    import types
    mod = types.ModuleType("antenv.axon_hooks")
    mod._hook = None
    mod.set_axon_ntff_profile_hook = lambda h: setattr(mod, "_hook", h)
    mod.get_axon_ntff_profile_hook = lambda: mod._hook
    sys.modules["antenv.axon_hooks"] = mod
    try:
        import antenv
        antenv.axon_hooks = mod
    except ImportError:
        pass
    try:
        from trn_agent_boot.trn_boot import _ntff_profile_via_ctypes
        mod._hook = _ntff_profile_via_ctypes("/opt/axon/libaxon_pjrt.so")
    except Exception:
        pass


def kernel(X, Wq, bq, Wk, bk, Wv, bv, Wo, bo):
    global last_exec_time_ns
    from concourse.bass_utils import run_bass_kernel_spmd
    _ensure_ntff_hook()

    X = np.asarray(X, dtype=np.float32)
    args = [np.asarray(a, dtype=np.float32)
            for a in (Wq, bq, Wk, bk, Wv, bv, Wo, bo)]

    if "nc" not in _cache:
        _cache["nc"] = _build()
    nc = _cache["nc"]

    in_maps = _prep_in_maps(X, *args)
    kwargs = {}
    tmpdir = os.environ.get("KERNEL_TRACE_DIR")
    if tmpdir:
        kwargs = dict(trace=True, tmpdir=tmpdir)
    try:
        res = run_bass_kernel_spmd(nc, in_maps,
                                   core_ids=list(range(N_CORES)), **kwargs)
    except Exception:
        if not kwargs and not os.environ.get("BASS_TRACE"):
            raise
        # trace post-processing can fail (no artifact share, old .so);
        # the numeric result must not depend on it
        os.environ["BASS_NEVER_TRACE"] = "1"
        try:
            res = run_bass_kernel_spmd(nc, in_maps,
                                       core_ids=list(range(N_CORES)))
        finally:
            del os.environ["BASS_NEVER_TRACE"]
    last_exec_time_ns = res.exec_time_ns

    out = np.empty((B, N, D), dtype=np.float32)
    for c in range(N_CORES):
        b, h = c // 2, c % 2
        out[b, h::2, :] = np.asarray(res.results[c]["out"],
                                     dtype=np.float32).T
    return out


# revision 21
# speedup vs baseline: 1.0351x; 1.0351x over previous
"""Causal self-attention (B=4, N=2048, D=1024, single head) on 8 TRN2 NeuronCores.

Sharding: core c handles batch b = c//2, query shard h = c%2 with the
stride-2 interleave q_global = 2*j + h  (j = 0..1023).  The interleave makes
the causal-mask *tile structure* identical on every core (SPMD-uniform), so
fully-masked score tiles are skipped structurally while the residual
diagonal masking is data-driven (query-position tensor per core).

Because the attention is single-head (D_head == D_model), the four weight
matrices fold into two host-side products, removing the K and V projections
entirely:
  scores ~ Xq @ (Wq^T Wk) @ X^T + (Wk^T bq).X^T   (+ per-query terms that
                                                   softmax ignores)
  out    = [P @ X] @ (Wo Wv)^T / rowsum + (bo + Wo bv)

Per-core pipeline (f32 PSUM accumulation everywhere):
  GT[e,q]  = W_qk^T @ Xq + bgt   ct-outer accumulation, one pass per query
                                 half, so matmuls start as soon as the first
                                 (wqk, xtq) DMA tiles land; gt tiles are
                                 split per query-half so ST jc0 does not
                                 depend on the second pass's evictions
  ST[k,j]  = X^T-pairs @ GT      (fp8 DoubleRow; half-width for the score
                                 tiles whose lower query half is fully
                                 masked -- their E halves are pre-zeroed)
  E        = exp(ST/sqrt(D)); boundary masks applied in-place on DVE
  rowsum[j]= ones.T @ E          (PE reduction over k partitions)
  Z[e,j]   = X^T @ E             (jc0 split into per-half column groups to
                                 skip the masked-out region; evict * 1/rowsum)
  OT[e,j]  = W_vo^T @ Z          per 512-chunk; Identity+bias eviction on the
                                 Act engine, fp16 output DMA on the Sync queue
                                 starting ~60% into the kernel

Scheduling rules learned from traces: DMA-issue instructions block the
issuing engine queue until the ring accepts them, so bulk loads only go on
queues ahead of that engine's compute needs (Act gets only what it needs
before its first exp); dependencies are tile-granular, so anything consumed
at finer granularity gets its own tile; dummy warm-up matmuls ramp the PE
DVFS clock during the initial DMA wait.  1/rowsum via reciprocal_approx_fast.
No collectives: each core receives exactly the host-side shard it needs.
"""

import os
import numpy as np
import ml_dtypes

BF16 = ml_dtypes.bfloat16
FP8 = ml_dtypes.float8_e4m3
F16 = np.float16

N_CORES = 8
B, N, D = 4, 2048, 1024
NQ = 1024           # queries per core
P = 128             # partitions
ET = D // P         # 8  e-tiles
CT_ = D // P        # 8  contraction tiles of D
KT_ALL = N // P     # 16 key tiles
JCW = 512           # query chunk
NJC = NQ // JCW     # 2
QCW = 256           # query half-chunk (causal granularity)

_cache = {}


def _build():
    from concourse import bacc, tile, mybir
    import concourse.bass as bass

    f32 = mybir.dt.float32
    bf16 = mybir.dt.bfloat16
    fp8 = mybir.dt.float8e4
    fp16 = mybir.dt.float16
    DR = mybir.MatmulPerfMode.DoubleRow
    Exp = mybir.ActivationFunctionType.Exp
    Ident = mybir.ActivationFunctionType.Identity
    is_ge = mybir.AluOpType.is_ge
    mult = mybir.AluOpType.mult
    PSUM = bass.MemorySpace.PSUM

    SCL = float(1.0 / np.sqrt(np.float32(D)))
    nc = bacc.Bacc("TRN2", target_bir_lowering=False, debug=False,
                   num_devices=N_CORES)

    xtp_d = nc.declare_dram_parameter("xtp", [ET // 2, P, 2, N], fp8,
                                      isOutput=False)
    xtq_d = nc.declare_dram_parameter("xtq", [D, NQ], bf16, isOutput=False)
    wqk_d = nc.declare_dram_parameter("wqk", [D, D], bf16, isOutput=False)
    wvot_d = nc.declare_dram_parameter("wvot", [D, D], bf16, isOutput=False)
    xtok_d = nc.declare_dram_parameter("xtok", [N, D], bf16, isOutput=False)
    bgt_d = nc.declare_dram_parameter("bgt", [P, ET], f32, isOutput=False)
    bot_d = nc.declare_dram_parameter("bot", [P, ET], f32, isOutput=False)
    bqp_d = nc.declare_dram_parameter("bqpos", [P, NQ], f32, isOutput=False)
    kpt_d = nc.declare_dram_parameter("kpost", [P, KT_ALL], f32, isOutput=False)
    out_d = nc.declare_dram_parameter("out", [D, NQ], fp16, isOutput=True)

    # Causal structure with the stride-2 query interleave: keys of tile kt
    # are visible only to query quarter-chunks qc >= kt//4.  Within a
    # 512-wide chunk jc, tiles whose lower quarter is fully masked are
    # computed half-width; the dead quarter of E is pre-zeroed instead.
    def dead_left(jc, kt):
        return kt >= 4 * (2 * jc + 1)

    with tile.TileContext(nc) as tc:
        with tc.tile_pool(name="sb", bufs=1) as p_sb:
            # gt tiles split per (d-pair, query-half) for fine-grained deps
            gtq = {(i, qh): p_sb.tile([P, 2, JCW], fp8,
                                      tag=f"qt{i}_{qh}", name="qt")
                   for i in range(ET // 2) for qh in range(NJC)}
            xtp_tiles = [p_sb.tile([P, 2, N], fp8, tag=f"kt{i}", name="kt")
                         for i in range(ET // 2)]
            xtok_tiles = [p_sb.tile([P, D], bf16, tag=f"v{k}", name="v")
                          for k in range(KT_ALL)]
            exps = {(jc, kt): p_sb.tile([P, JCW], bf16,
                                        tag=f"exp{jc}_{kt}", name="exp")
                    for jc in range(NJC) for kt in range(KT_ALL)
                    if kt < (8 if jc == 0 else 16)}

            # t=0 work on DVE: constants + zero the dead E halves so the
            # full-width rowsum matmuls read zeros there
            ones_col = p_sb.tile([P, 1], bf16, tag="ones_col")
            nc.vector.memset(ones_col[:], 1.0)
            ones_row = p_sb.tile([1, P], bf16, tag="ones_row")
            nc.vector.memset(ones_row[:], 1.0)
            dummy_b = p_sb.tile([P, JCW], bf16, tag="dummy")
            nc.vector.memset(dummy_b[:], 1.0)
            for jc in range(NJC):
                for kt in range(KT_ALL):
                    if (jc, kt) in exps and dead_left(jc, kt):
                        nc.vector.memset(exps[(jc, kt)][:, :QCW], 0.0)

            # ---- DMA issue plan.  Act (scalar) only gets what must land
            # before its first compute; Sync carries the early weights and
            # half the late bulk + outputs; gpsimd (software DGE) the rest.
            wq = [p_sb.tile([P, D], bf16, tag=f"wq{c}", name="wq")
                  for c in range(CT_)]
            xtq_tiles = [p_sb.tile([P, NQ], bf16, tag=f"xtq{c}",
                                   name="xtq") for c in range(CT_)]
            wo = [p_sb.tile([P, D], bf16, tag=f"wo{c}", name="wo")
                  for c in range(CT_)]

            def load(eng, t, dram, r0):
                eng.dma_start(t[:], dram[r0 * P:(r0 + 1) * P, :])

            bgt_t = p_sb.tile([P, ET], f32, tag="bgt")
            nc.sync.dma_start(bgt_t[:], bgt_d[:, :])
            qs = [nc.sync, nc.scalar, nc.sync, nc.scalar, nc.sync,
                  nc.scalar, nc.sync, nc.scalar, nc.sync, nc.scalar,
                  nc.sync, nc.scalar, nc.sync, nc.scalar, nc.gpsimd,
                  nc.gpsimd]
            for ct in range(CT_):
                load(qs[2 * ct], wq[ct], wqk_d, ct)
                load(qs[2 * ct + 1], xtq_tiles[ct], xtq_d, ct)
            for i in range(ET // 2):
                eng = nc.sync if i < 2 else nc.gpsimd
                eng.dma_start(xtp_tiles[i][:], xtp_d[i])
            bqpos_t = p_sb.tile([P, NQ], f32, tag="bqpos")
            nc.gpsimd.dma_start(bqpos_t[:], bqp_d[:, :])
            kpost_t = p_sb.tile([P, KT_ALL], f32, tag="kpost")
            nc.gpsimd.dma_start(kpost_t[:], kpt_d[:, :])
            for kt in range(KT_ALL):
                eng = nc.sync if kt % 2 == 0 else nc.gpsimd
                load(eng, xtok_tiles[kt], xtok_d, kt)
            for ct in range(CT_):
                eng = nc.sync if ct % 2 == 0 else nc.gpsimd
                load(eng, wo[ct], wvot_d, ct)
            bot_t = p_sb.tile([P, ET], f32, tag="bot")
            nc.gpsimd.dma_start(bot_t[:], bot_d[:, :])

            # ---- GT = W_qk^T @ Xq + bgt : ct-outer accumulation in
            # PSUM so matmuls start as soon as (wq[ct], xtq[ct]) land.
            # One pass per query-half: the qh=0 evictions finish during
            # the qh=1 compute pass, so ST jc0 starts with zero stall ----
            with tc.tile_pool(name="gtps", bufs=8, space=PSUM) as p_gt:
                # dummy matmuls ramp the PE clock while the first DMAs land
                dps = p_gt.tile([P, JCW], f32, tag="gtps", name="gtps")
                for _ in range(40):
                    nc.tensor.matmul(dps[0:1, :], ones_col[:, :],
                                     dummy_b[:], start=True, stop=True)
                for qh in range(NJC):
                    gtps = [p_gt.tile([P, JCW], f32, tag="gtps",
                                      name="gtps") for _ in range(ET)]
                    for ct in range(CT_):
                        for et in range(ET):
                            nc.tensor.matmul(
                                gtps[et][:],
                                wq[ct][:, et * P:(et + 1) * P],
                                xtq_tiles[ct][:, qh * JCW:(qh + 1) * JCW],
                                start=(ct == 0), stop=(ct == CT_ - 1))
                    for et in range(ET):
                        dst = gtq[(et // 2, qh)][:, et % 2, :]
                        if et % 2 == 0:
                            nc.scalar.activation(dst, gtps[et][:], Ident,
                                                 bias=bgt_t[:, et:et + 1])
                        else:
                            nc.vector.tensor_scalar_add(
                                dst, gtps[et][:], bgt_t[:, et:et + 1])

            with (
                tc.tile_pool(name="ps", bufs=2, space=PSUM) as p_ps,
                tc.tile_pool(name="ot", bufs=4, space=PSUM) as p_ot,
                tc.tile_pool(name="rsps", bufs=2, space=PSUM) as p_rs,
            ):
                # jc=0 covers global queries [0,1024): keys < 1024 (kt 0..7).
                # jc=1 covers [1024,2048): all 16 kt.
                rs_ps = {}
                nkt_of = {0: ET, 1: KT_ALL}

                def st_block(jc):
                    """ST + exp + mask + rowsum for one 512-wide chunk."""
                    nkt = nkt_of[jc]
                    rs_ps[jc] = p_rs.tile([1, JCW], f32, tag="rsps",
                                          name="rsps")
                    pend = []

                    def issue_rowsum(kt):
                        nc.tensor.matmul(
                            rs_ps[jc][:], ones_col[:], exps[(jc, kt)][:],
                            start=(kt == 0), stop=(kt == nkt - 1))

                    for kt in range(nkt):
                        dead = dead_left(jc, kt)
                        csl = slice(QCW, JCW) if dead else slice(0, JCW)
                        sts = p_ps.tile([P, JCW], f32, tag="ps", name="ps")
                        for i in range(ET // 2):
                            nc.tensor.matmul(
                                sts[:, csl],
                                xtp_tiles[i][:, :, kt * P:(kt + 1) * P],
                                gtq[(i, jc)][:, :, csl],
                                start=(i == 0), stop=(i == ET // 2 - 1),
                                perf_mode=DR)
                        ex_t = exps[(jc, kt)]
                        nc.scalar.activation(ex_t[:, csl], sts[:, csl],
                                             Exp, scale=SCL)
                        # in-place causal mask on the boundary quarter
                        for qh in range(2):
                            qc = 2 * jc + qh
                            if not (4 * qc <= kt < 4 * (qc + 1)):
                                continue
                            esl = slice(qh * QCW, (qh + 1) * QCW)
                            jsl = slice(jc * JCW + qh * QCW,
                                        jc * JCW + (qh + 1) * QCW)
                            nc.vector.scalar_tensor_tensor(
                                ex_t[:, esl], bqpos_t[:, jsl],
                                kpost_t[:, kt:kt + 1], ex_t[:, esl],
                                is_ge, mult)
                        pend.append(kt)
                        if len(pend) > 2:
                            issue_rowsum(pend.pop(0))
                    for kt in pend:
                        issue_rowsum(kt)

                def recip_block(jc):
                    r32 = p_sb.tile([1, JCW], f32, tag=f"r32_{jc}",
                                    name="recip32")
                    nc.vector.reciprocal_approx_fast(r32[:], rs_ps[jc][:])
                    r16 = p_sb.tile([1, JCW], bf16, tag=f"r16_{jc}",
                                    name="recip16")
                    nc.vector.tensor_copy(r16[:], r32[:])
                    return r16

                brec = {}

                def brec_block(jc, r16):
                    # broadcast 1/rowsum across partitions via K=1 matmul,
                    # then park it in SBUF so the PSUM bank frees right away
                    bps = p_ps.tile([P, JCW], f32, tag="ps", name="ps")
                    nc.tensor.matmul(bps[:], ones_row[:], r16[:],
                                     start=True, stop=True)
                    bt = p_sb.tile([P, JCW], f32, tag=f"brec{jc}",
                                   name="brec")
                    nc.vector.tensor_copy(bt[:], bps[:])
                    brec[jc] = bt

                zs = {}

                def zot_block(jc):
                    """Z then OT for one 512-chunk, with OT's first et-half
                    interleaved ct-outer into the Z loop so the PE never
                    waits on Z evictions, and output DMA starts early."""
                    jsl = slice(jc * JCW, (jc + 1) * JCW)
                    nkt = nkt_of[jc]
                    ot_ps = {}

                    def ot_mms(ct, ets):
                        for et in ets:
                            nc.tensor.matmul(
                                ot_ps[et][:],
                                wo[ct][:, et * P:(et + 1) * P],
                                zs[(jc, ct)][:],
                                start=(ct == 0), stop=(ct == CT_ - 1))

                    def ot_evict(ets):
                        for et in ets:
                            of = p_sb.tile([P, JCW], fp16,
                                           tag=f"of{jc}_{et}", name="of")
                            nc.scalar.activation(of[:], ot_ps[et][:], Ident,
                                                 bias=bot_t[:, et:et + 1])
                            eng = nc.sync if et % 2 == 0 else nc.gpsimd
                            eng.dma_start(
                                out_d[et * P:(et + 1) * P, jsl], of[:])

                    ets1, ets2 = (0, 1, 2, 3), (4, 5, 6, 7)
                    for et in ets1:
                        ot_ps[et] = p_ot.tile([P, JCW], f32, tag="ot",
                                              name="ot")
                    for ct in range(CT_):
                        zps = p_ps.tile([P, JCW], f32, tag="ps", name="ps")
                        for kt in range(nkt):
                            csl = (slice(QCW, JCW) if dead_left(jc, kt)
                                   else slice(0, JCW))
                            nc.tensor.matmul(
                                zps[:, csl],
                                xtok_tiles[kt][:, ct * P:(ct + 1) * P],
                                exps[(jc, kt)][:, csl],
                                start=(kt == 0), stop=(kt == nkt - 1),
                                skip_group_check=True)
                        z_t = p_sb.tile([P, JCW], bf16,
                                        tag=f"z{jc}_{ct}", name="z")
                        nc.vector.tensor_tensor(z_t[:], zps[:],
                                                brec[jc][:], mult)
                        zs[(jc, ct)] = z_t
                        if ct >= 1:
                            ot_mms(ct - 1, ets1)
                    ot_mms(CT_ - 1, ets1)
                    ot_evict(ets1)
                    for et in ets2:
                        ot_ps[et] = p_ot.tile([P, JCW], f32, tag="ot",
                                              name="ot")
                    for ct in range(CT_):
                        ot_mms(ct, ets2)
                    ot_evict(ets2)

                # ---- PE program order; recip chains overlap the next block
                st_block(0)
                recip0 = recip_block(0)
                st_block(1)
                brec_block(0, recip0)
                recip1 = recip_block(1)
                brec_block(1, recip1)
                zot_block(0)
                zot_block(1)

    nc.compile()
    return nc


def _prep_in_maps(X, Wq, bq, Wk, bk, Wv, bv, Wo, bo):
    wqk = np.ascontiguousarray(Wq.astype(np.float64).T
                               @ Wk.astype(np.float64)).astype(BF16)
    wvot = np.ascontiguousarray((Wo.astype(np.float64)
                                 @ Wv.astype(np.float64)).T).astype(BF16)
    bgt = np.ascontiguousarray(
        (Wk.astype(np.float64).T @ bq.astype(np.float64))
        .reshape(ET, P).T).astype(np.float32)
    bo_eff = (bo.astype(np.float64)
              + Wo.astype(np.float64) @ bv.astype(np.float64))
    bot = np.ascontiguousarray(
        bo_eff.reshape(ET, P).T).astype(np.float32)
    kpost = np.ascontiguousarray(
        np.arange(N, dtype=np.float32).reshape(KT_ALL, P).T)

    in_maps = []
    for c in range(N_CORES):
        b, h = c // 2, c % 2
        Xb = X[b]
        xtok = np.ascontiguousarray(Xb).astype(BF16)
        xtq = np.ascontiguousarray(Xb[h::2].T).astype(BF16)
        xtp = np.ascontiguousarray(
            Xb.T.reshape(ET // 2, 2, P, N).transpose(0, 2, 1, 3)
        ).astype(FP8)
        qpos = (2.0 * np.arange(NQ, dtype=np.float32) + h)
        bqpos = np.ascontiguousarray(
            np.broadcast_to(qpos[None, :], (P, NQ))).astype(np.float32)
        in_maps.append({
            "xtp": xtp, "xtq": xtq, "xtok": xtok,
            "wqk": wqk, "wvot": wvot,
            "bgt": bgt, "bot": bot,
            "bqpos": bqpos, "kpost": kpost,
        })
    return in_maps


last_exec_time_ns = None


def _ensure_ntff_hook():
    """Register the axon NTFF profile hook if the image's antenv lacks it."""
    try:
        from antenv.axon_hooks import get_axon_ntff_profile_hook  # noqa: F401
        return
    except ImportError:
        pass
    import sys
    import types
    mod = types.ModuleType("antenv.axon_hooks")
    mod._hook = None
    mod.set_axon_ntff_profile_hook = lambda h: setattr(mod, "_hook", h)
    mod.get_axon_ntff_profile_hook = lambda: mod._hook
    sys.modules["antenv.axon_hooks"] = mod
    try:
        import antenv
        antenv.axon_hooks = mod
    except ImportError:
        pass
    try:
        from trn_agent_boot.trn_boot import _ntff_profile_via_ctypes
        mod._hook = _ntff_profile_via_ctypes("/opt/axon/libaxon_pjrt.so")
    except Exception:
        pass


def kernel(X, Wq, bq, Wk, bk, Wv, bv, Wo, bo):
    global last_exec_time_ns
    from concourse.bass_utils import run_bass_kernel_spmd
    _ensure_ntff_hook()

    X = np.asarray(X, dtype=np.float32)
    args = [np.asarray(a, dtype=np.float32)
            for a in (Wq, bq, Wk, bk, Wv, bv, Wo, bo)]

    if "nc" not in _cache:
        _cache["nc"] = _build()
    nc = _cache["nc"]

    in_maps = _prep_in_maps(X, *args)
    kwargs = {}
    tmpdir = os.environ.get("KERNEL_TRACE_DIR")
    if tmpdir:
        kwargs = dict(trace=True, tmpdir=tmpdir)
    try:
        res = run_bass_kernel_spmd(nc, in_maps,
                                   core_ids=list(range(N_CORES)), **kwargs)
    except Exception:
        if not kwargs and not os.environ.get("BASS_TRACE"):
            raise
        # trace post-processing can fail (no artifact share, old .so);
        # the numeric result must not depend on it
        os.environ["BASS_NEVER_TRACE"] = "1"
        try:
            res = run_bass_kernel_spmd(nc, in_maps,
                                       core_ids=list(range(N_CORES)))
        finally:
            del os.environ["BASS_NEVER_TRACE"]
    last_exec_time_ns = res.exec_time_ns

    out = np.empty((B, N, D), dtype=np.float32)
    for c in range(N_CORES):
        b, h = c // 2, c % 2
        out[b, h::2, :] = np.asarray(res.results[c]["out"],
                                     dtype=np.float32).T
    return out
